# revision 11
# baseline (speedup 1.0000x reference)
"""Trainium2 Bass kernel for nn_Conv_89713276879316.

Reference semantics (faithful channel bug): take ONLY the last channel of
image [32, 3, 512, 512], zero-pad by 7, cross-correlate with the 15x15
kernel, broadcast the [32, 1, 512, 512] result to all 3 channels.

Strategy (16-way PE array tiling):
  - Host: extract channel 2, zero-pad to [4, 526, 526] bf16 per core
    (data-parallel: 4 images per core across 8 cores).
  - Device: the 128x128 PE array runs as 16 independent 32x32 subarrays
    (tile_position).  Subarray (i, j) convolves one 32-row window of a
    padded image (rows at SBUF partitions [32i, 32i+32), data at free
    block j) with 15 accumulating matmuls (one per kernel column dx, a
    free-dim offset), yielding 18 output rows in PSUM bank i partitions
    [32j, 32j+18).  116 windows (stride 18) run in chunks of 16.
  - Orderings that matter (each measured):
    * MM issue (dx, t) with row group i = t % 4 fastest: consecutive MMs
      hit different row groups so LDWEIGHTS pulls ahead of in-flight MMs
      through the PE reorder window (12x vs naive).
    * All window input DMAs issue up front: input-wait lane ticks on the
      shared DMAHW semaphores can never capture an out-DMA tick.
    * Output DMAs go via nc.gpsimd (Pool -> SWDGE + DMASW lanes), fully
      decoupled from input DMAHW lanes (no head-of-line blocking).
  - PSUM evacuated by 4 full-bank DVE copies per chunk, f32 -> bf16
    staging (halves DVE cost and out-DMA bytes); host upcasts to f32.
  - Accuracy: bf16 in/out, f32 PSUM accumulate -> rel err ~2e-3 (gate
    2e-2).  Measured ~53 us/body vs ~120 us for the f32r full-array
    banded baseline on the same repeat-slope metric.
"""

import sys

import numpy as np

try:
    import concourse.bass as bass
except ImportError:  # pragma: no cover - fallback path inside the container
    sys.path.insert(0, "/opt/trn_rl_repo")
    import concourse.bass as bass

import ml_dtypes
from contextlib import ExitStack

import concourse.tile as tile
from concourse import bacc, mybir
from concourse.bass_utils import run_bass_kernel_spmd

N_CORES = 8
N_IMG = 32
C_IMG = 3
H = W = 512
KS = 15
PAD = KS // 2  # 7
HP = H + 2 * PAD  # 526
PER_CORE = N_IMG // N_CORES  # 4
WROWS = 32  # window rows (matmul contract dim)
OROWS = 18  # output rows per window (WROWS - KS + 1)
CHUNK = 16  # windows in flight = number of 32x32 PE subarrays

# window start rows: stride OROWS, last window clamped so rows fit in HP
SLIST = list(range(0, H - OROWS + 1, OROWS))
if SLIST[-1] != H - OROWS:
    SLIST.append(H - OROWS)  # 494: overlaps previous window, same values

F32 = mybir.dt.float32
BF16 = mybir.dt.bfloat16

_CACHE = {}


def _build_nc(repeat=1, loop=False, timing=False, skip_in=False, skip_out=False, skip_dma_out=False, copy_eng='dve'):
    """Per-core Bass program (identical on all 8 cores).

    timing=True: large tensors DRAM-Internal (garbage data, no host
    transfer) plus tiny tick/tock externals, so wall time is RPC floor +
    device time; repeat/loop re-run the body for slope timing.
    """
    nc = bacc.Bacc("TRN2", target_bir_lowering=False, debug=False)

    big = "Internal" if timing else None
    img = nc.dram_tensor(
        "img", [PER_CORE, HP, HP], BF16, kind=big or "ExternalInput"
    ).ap()
    bands = nc.dram_tensor(
        "bands", [128, KS * 32], BF16, kind=big or "ExternalInput"
    ).ap()
    out = nc.dram_tensor(
        "out", [PER_CORE, H, W], BF16, kind=big or "ExternalOutput"
    ).ap()
    if timing:
        tick = nc.dram_tensor("tick", [1, 16], F32, kind="ExternalInput").ap()
        tock = nc.dram_tensor("tock", [1, 16], F32, kind="ExternalOutput").ap()

    worklist = [(m, s) for m in range(PER_CORE) for s in SLIST]
    chunks = [worklist[c : c + CHUNK] for c in range(0, len(worklist), CHUNK)]

    with tile.TileContext(nc) as tc, ExitStack() as ctx:
        const_pool = ctx.enter_context(tc.tile_pool(name="const", bufs=1))
        win_pool = ctx.enter_context(tc.tile_pool(name="winp", bufs=8))
        psum_pool = ctx.enter_context(tc.tile_pool(name="psum", bufs=2, space="PSUM"))
        out_pool = ctx.enter_context(tc.tile_pool(name="outp", bufs=2))

        bands_sb = const_pool.tile([128, KS * 32], BF16)
        nc.sync.dma_start(bands_sb[:], bands[:, :])
        if timing:
            tpool = ctx.enter_context(tc.tile_pool(name="tickp", bufs=1))
            tt = tpool.tile([1, 16], F32)
            nc.sync.dma_start(tt[:], tick[:, :])

        def body(_iv=None):
            # preload every chunk's windows up front: all input DMAs get
            # low ticks on the shared DMA semaphore lanes, so no matmul
            # wait can transitively capture an out-DMA tick
            wins = []
            for chunk in chunks:
                win = win_pool.tile([128, 4 * HP], BF16, name="win", tag="win")
                if not skip_in:
                    for t, (m, s) in enumerate(chunk):
                        i, j = t % 4, t // 4
                        nc.sync.dma_start(
                            win[32 * i : 32 * i + WROWS, j * HP : j * HP + HP],
                            img[m, s : s + WROWS, :],
                        )
                wins.append(win)

            for ci, chunk in enumerate(chunks):
                n = len(chunk)
                win = wins[ci]
                pb = [
                    psum_pool.tile([128, 512], F32, name="pb", tag=f"pb{i}")
                    for i in range(4)
                ]
                # i = t % 4 fastest: consecutive MMs hit different row
                # groups so next LDWEIGHTS overlaps in-flight MMs
                for dx in range(KS):
                    for t in range(n):
                        i, j = t % 4, t // 4
                        nc.tensor.matmul(
                            pb[i][32 * j : 32 * j + OROWS, :W],
                            bands_sb[32 * i : 32 * i + WROWS, dx * 32 : dx * 32 + OROWS],
                            win[32 * i : 32 * i + WROWS, j * HP + dx : j * HP + dx + W],
                            start=(dx == 0),
                            stop=(dx == KS - 1),
                            tile_position=(32 * i, 32 * j),
                        )
                if skip_out:
                    continue
                nbank = min(n, 4)
                st = [
                    out_pool.tile([128, 512], BF16, name="st", tag=f"st{i}")
                    for i in range(nbank)
                ]
                for i in range(nbank):
                    if copy_eng == 'dve':
                        nc.vector.tensor_copy(st[i][:, :], pb[i][:, :])
                    elif copy_eng == 'act':
                        nc.scalar.copy(st[i][:, :], pb[i][:, :])
                    else:
                        (nc.vector.tensor_copy if i % 2 == 0 else nc.scalar.copy)(
                            st[i][:, :], pb[i][:, :]
                        )
                if skip_dma_out:
                    continue
                for t, (m, s) in enumerate(chunk):
                    i, j = t % 4, t // 4
                    # gpsimd = Pool engine -> SWDGE queue + DMASW semaphore
                    # lanes, fully decoupled from the input DMAHW lanes
                    nc.gpsimd.dma_start(
                        out[m, s : s + OROWS, :],
                        st[i][32 * j : 32 * j + OROWS, :W],
                    )

        if loop and repeat > 1:
            with tc.For_i(0, repeat, 1):
                body()
        else:
            for _ in range(repeat):
                body()

        if timing:
            nc.sync.dma_start(tock[:, :], tt[:])

    nc.compile()
    return nc


def _prep_inputs(image: np.ndarray, kernel: np.ndarray):
    """Host-side prep: channel select, pad, bf16 cast, band matrix."""
    ch = np.ascontiguousarray(image[:, -1, :, :]).astype(np.float32)
    padded = np.zeros((N_IMG, HP, HP), np.float32)
    padded[:, PAD : PAD + H, PAD : PAD + W] = ch
    planes = padded.astype(ml_dtypes.bfloat16)

    w = kernel.astype(np.float32)
    # bands[r, dx*32 + y] = w[r - y, dx] for 0 <= r-y < KS, y < OROWS
    b = np.zeros((32, KS, 32), np.float32)
    for y in range(OROWS):
        for r in range(y, min(y + KS, WROWS)):
            b[r, :, y] = w[r - y, :]
    bands128 = np.tile(b.reshape(32, KS * 32), (4, 1)).astype(ml_dtypes.bfloat16)
    return planes, bands128


def kernel(image: np.ndarray, kernel: np.ndarray) -> np.ndarray:
    planes, bands128 = _prep_inputs(image, kernel)

    if "nc" not in _CACHE:
        _CACHE["nc"] = _build_nc()
    nc = _CACHE["nc"]

    in_maps = []
    for c in range(N_CORES):
        s = slice(c * PER_CORE, (c + 1) * PER_CORE)
        in_maps.append({"img": planes[s], "bands": bands128})

    res = run_bass_kernel_spmd(nc, in_maps, core_ids=list(range(N_CORES)))
    _CACHE["last_results"] = res

    full = np.concatenate(
        [res.results[c]["out"].astype(np.float32) for c in range(N_CORES)], axis=0
    )
    out = np.broadcast_to(full[:, None, :, :], (N_IMG, C_IMG, H, W))
    return np.ascontiguousarray(out)
